# revision 40
# baseline (speedup 1.0000x reference)
"""Single-head causal attention (V=K source bug) on 8 trn2 NeuronCores.

Problem: x[4,2048,1024], W_Q/W_K/W_V[64,1024] (W_V unused by reference).
  Q = x @ W_Q.T ; K = x @ W_K.T ; V = K (reference bug)
  out = softmax(mask(Q K^T / sqrt(1024))) @ V      -> [4,2048,64]

Sharding: 2 cores per batch (core i: batch = i % 4, role r = i // 4).
Each batch's 8 query tiles of 256 rows split by parity (r=0 even, r=1 odd).
ONE SPMD graph for all 8 cores. Per-core differences are folded into DATA:

 * x^T is sent column-PERMUTED, own query tiles first:
     positions 0..3 = own tiles (2j+r), positions 4..7 = other tiles.
   Causality over the permuted key order is encoded in per-core 0/1 masks.
 * slot j (own tile 2j+r) attends own chunks [0..2j+1] and other chunks
   [8..8+2j+1] (uniform r=1 shape; r=0 masks the padded tail).

Pipeline (emission strictly follows slab arrival; one NEFF, no
collectives -- their latency floor exceeds the whole kernel):
 * 8 half-slab DMAs (4KB/partition) on the sync HWDGE queue; the per-ring
   FIFO staggers slab completions ~2.4us apart at ~400GB/s aggregate. The
   w tensor rides the ACT HWDGE queue in parallel. The Pool queue is a
   SOFTWARE DGE (3-7us/transfer) and only carries scal.
 * Combined [K|Q] projection for slabs 0/1: lhsT = w[:,c,0:128] covers
   W_K (psum rows 0-63 -> K-stack top) and W_Q (rows 64-127 -> qT[64:128],
   the bottom-stack S^T rhs) in one 512-col chain -- half the PE time of
   separate chains. Slabs 2/3 run K-only chains into rows 64-127.
 * qT[0:64] (top-stack rhs; HW requires lhsT/rhs on the same partitions):
   quarter 1 via an extra M=64 chain + ACT-engine copy (chain head),
   quarters 2-3 via one [64,512] chain, quarter 0 via a sync-queue dup
   DMA (needed latest).
 * Per 4-chunk group: S^T matmuls (mixed groups alternate PE array halves
   AND psum banks -- two matmuls in the same bank cannot overlap), exp on
   ScalarE ([128,1024] PSUM->SBUF bf16, 1/sqrt(C) folded into the
   activation scale; no max-subtraction -- |scores| <= ~1), causal masks
   as 0/1 MULs on DVE. The serial ~11us exp chain + the ~18us saturated
   PE queue are the joint critical path; warmup matmuls cover the DMA
   wait so the DVFS boost (a ~3.4us activity window; idle gaps drop the
   clock 2x) is up before the projections run.
 * V natural (V=K) via PE transposes of K^T; PV matmuls are CLOSED 4-chain
   psum groups with lhsT=[V|ones] so psum row 64 accumulates the softmax
   denominator; phase-2/3 PV chains carry an explicit dep behind the next
   projection chain so the scheduler cannot starve the exp chain. Each
   slot ships its [65,256] output as soon as its last PV lands.
 * Host divides by row 64 and transposes the [65,1024] outputs back.
 * Fixed overheads (measured window): ~1.3us framework preamble before
   the tile barrier and a ~10us postamble (per-engine serial reset of the
   whole 254-semaphore file at ~115ns each) -- both framework-emitted.
"""

import os
import sys

sys.path.insert(0, "/opt/trn_rl_repo")

import numpy as np
import ml_dtypes

BF16 = ml_dtypes.bfloat16

B, T, C, D = 4, 2048, 1024, 64
N_CORES = 8
QTILE = 256          # query rows per slot
N_SLOTS = 4
CHUNK = 128          # key chunk
GROUP = 4            # chunks per exp group ([128, 4*256] psum tile)
SCALE = C ** -0.5
N_WARMUP = 52        # HAM warmup matmuls (cover the DMA wait before proj0)

TRACE = False
TRACE_CORES = None
LAST_RESULTS = None


def _slot_groups_def(j):
    """Groups of 4 chunks for slot j with a mask kind per group.

    Two shapes are mixed for pipeline reasons:
      * (own, oth) PAIRED groups [own_a, oth_a, own_b, oth_b]: the own
        matmul (PE rows 0-63) and the oth matmul (rows 64-127) overlap on
        the array halves, halving S^T time -- but they need a bottom slab.
      * UNPAIRED groups ({0..3} / {8..11} / mixed) for slots 1-2 so the
        exp chain can start from slab 0/1 alone.
    Kinds: 'pair' (no mask), 'pair_last' (slices 0,2 diag MUL; 1,3 scal),
    'own_diag' (slices 2-3 MUL), 'oth_tail' (slices 2-3 scal), 'mixed'
    (slices 0-1 MUL, 2-3 scal), 'plain'."""
    if j == 0:
        return [([0, 1, 8, 9], "mixed")]
    if j == 1:
        return [([0, 1, 2, 3], "own_diag"), ([8, 9, 10, 11], "oth_tail")]
    if j == 2:
        return [
            ([0, 1, 2, 3], "plain"),
            ([8, 9, 10, 11], "plain"),
            ([4, 5, 12, 13], "mixed"),
        ]
    return [
        ([0, 1, 2, 3], "plain"),
        ([8, 9, 10, 11], "plain"),
        ([4, 5, 6, 7], "own_diag"),
        ([12, 13, 14, 15], "oth_tail"),
    ]


def _chunk_stack(c):
    """abs permuted chunk c -> (stack_idx, half, within). Stack A covers
    permuted cols 0-511 (top) and 1024-1535 (bottom); B covers 512-1023
    (top) and 1536-2047 (bottom)."""
    pos = c // 2            # 256-col tile position 0..7
    if pos < 4:             # own side -> top halves
        return (pos // 2, 0, c % 4)
    else:                   # other side -> bottom halves
        return ((pos - 4) // 2, 1, c % 4)


def _build_graph():
    import concourse.bass as bass
    import concourse.mybir as mybir
    import concourse.tile as tile
    from concourse.tile import add_dep_helper
    from concourse import bacc
    from concourse.masks import make_identity
    from contextlib import ExitStack

    fp32 = mybir.dt.float32
    bf16 = mybir.dt.bfloat16

    nc = bacc.Bacc(
        "TRN2",
        target_bir_lowering=False,
        debug=False,
        num_devices=N_CORES,
    )

    # slab-major layout: per-partition runs are 8KB contiguous, one DMA
    # per slab (128 descriptors)
    x4 = nc.dram_tensor("x4", [4, 128, C // CHUNK, 512], bf16,
                        kind="ExternalInput").ap()
    wkq = nc.dram_tensor("wkq", [128, C // CHUNK, 2 * D], bf16,
                         kind="ExternalInput").ap()
    scald = nc.dram_tensor(
        "scal", [CHUNK, N_SLOTS], fp32, kind="ExternalInput"
    ).ap()
    out = nc.dram_tensor(
        "out", [D + 1, N_SLOTS * QTILE], fp32, kind="ExternalOutput"
    ).ap()

    NQ = N_SLOTS * QTILE           # 1024 own query cols
    CCH = C // CHUNK               # 8 contraction chunks

    with tile.TileContext(nc) as tc, ExitStack() as ctx:
        consts = ctx.enter_context(tc.tile_pool(name="consts", bufs=1))
        xpool = ctx.enter_context(tc.tile_pool(name="xpool", bufs=1))
        kqpool = ctx.enter_context(tc.tile_pool(name="kqpool", bufs=1))
        ptpool = ctx.enter_context(tc.tile_pool(name="ptpool", bufs=10))
        psP = ctx.enter_context(tc.tile_pool(name="psP", bufs=2, space="PSUM"))
        psS = ctx.enter_context(tc.tile_pool(name="psS", bufs=2, space="PSUM"))
        psO = ctx.enter_context(tc.tile_pool(name="psO", bufs=2, space="PSUM"))

        # ---- constants ----
        # warmup matmuls on a memset tile: near-zero deps, start immediately
        warm_src = consts.tile([128, 128], bf16)
        nc.vector.memset(warm_src, 0.0)
        warm_ps = psP.tile([128, 128], fp32, tag="proj")
        for w in range(N_WARMUP):
            nc.tensor.matmul(
                warm_ps, lhsT=warm_src, rhs=warm_src,
                start=(w == 0), stop=(w == N_WARMUP - 1),
            )
        ident = consts.tile([128, 128], bf16)
        make_identity(nc, ident)
        warm = consts.tile([1, 1], fp32)
        nc.vector.memset(warm, 0.0)
        nc.scalar.activation(warm, warm, mybir.ActivationFunctionType.Exp)

        # ---- DMAs (slab order drives the pipeline) ----
        # NOTE: only the sync queue is a hardware DGE here; the Pool queue
        # is a software DGE (~3-7us per transfer) -- keep it off the
        # critical path (scal only). w rides the ACT HW queue in parallel
        # with the slab descgens on sync.
        w_sb = consts.tile([128, CCH, 2 * D], bf16)
        nc.scalar.dma_start(out=w_sb, in_=wkq)
        # x slabs: 4 x [128, CCH, 512] column slabs of the permuted x^T,
        # two half-slab DMAs each (4KB/partition) on the sync queue: the
        # per-ring FIFO staggers them, and the projection chains can start
        # on cchunks 0-3 while 4-7 are still in flight
        xs = []
        for s in range(4):
            xsl = xpool.tile([128, CCH, 512], bf16, name=f"xslab{s}")
            xs.append(xsl)
            nc.sync.dma_start(out=xsl[:, 0:4, :], in_=x4[s, :, 0:4, :])
            nc.sync.dma_start(out=xsl[:, 4:8, :], in_=x4[s, :, 4:8, :])
        scal_sb = consts.tile([128, N_SLOTS], fp32)
        nc.gpsimd.dma_start(out=scal_sb, in_=scald)
        # causal diag mask, identical for every slot/core (built on-device):
        # cols 0:256 valid iff p <= f; cols 256:512 valid iff p+128 <= f
        mask_sb = consts.tile([128, 2 * QTILE], bf16)
        nc.gpsimd.memset(mask_sb, 1.0)
        nc.gpsimd.affine_select(
            out=mask_sb[:, 0:QTILE], in_=mask_sb[:, 0:QTILE],
            compare_op=mybir.AluOpType.is_ge, fill=0.0,
            base=0, channel_multiplier=-1, pattern=[[1, QTILE]],
        )
        nc.gpsimd.affine_select(
            out=mask_sb[:, QTILE:], in_=mask_sb[:, QTILE:],
            compare_op=mybir.AluOpType.is_ge, fill=0.0,
            base=-CHUNK, channel_multiplier=-1, pattern=[[1, QTILE]],
        )

        # ---- SBUF state ----
        qT = kqpool.tile([128, NQ], bf16)   # Q^T duplicated in both halves
        kstk = []
        vones = []
        o_done = {j: False for j in range(N_SLOTS)}
        o_all = kqpool.tile([D + 1, N_SLOTS, QTILE], fp32)

        for si in range(2):
            kt = kqpool.tile([128, 512], bf16, name=f"kstk{si}")
            kstk.append(kt)
            vo = kqpool.tile([128, 8, D + 1], bf16, name=f"vones{si}")
            nc.vector.memset(vo[:, :, D : D + 1], 1.0)
            vones.append(vo)

        def filler(n, tag):
            f_ps = psP.tile([128, 128], fp32, tag="proj", name=f"warmf_{tag}")
            for w in range(n):
                nc.tensor.matmul(
                    f_ps, lhsT=warm_src, rhs=warm_src,
                    start=(w == 0), stop=(w == n - 1),
                )

        def proj_top(si):
            """Combined [K|Q] projection for top slab si (si=0 -> slab0,
            si=1 -> slab1): one 8-matmul chain; psum rows 0-63 = K-top,
            rows 64-127 = Q quarters 2si..2si+1 (bottom-stack S^T rhs)."""
            slab = xs[si]
            kq_ps = psP.tile([128, 512], fp32, tag="proj", name=f"kqps{si}")
            for c in range(CCH):
                mi = nc.tensor.matmul(
                    kq_ps, lhsT=w_sb[:, c, 0 : 2 * D], rhs=slab[:, c, :],
                    start=(c == 0), stop=(c == CCH - 1),
                )
                if c == 0:
                    pe_after_sexp(mi)
            cs = slice(si * 512, si * 512 + 512)
            nc.vector.tensor_copy(kstk[si][0:64, :], kq_ps[0:64, :])
            nc.vector.tensor_copy(qT[64:128, cs], kq_ps[64:128, :])

        def proj_q(q, width=256):
            """Extra M=64 Q chain for quarter(s) starting at q into
            partitions 0-63 (the top-stack S^T rhs). Quarter 0 instead
            rides a dup DMA (slack)."""
            slab = xs[q // 2]
            coff = (q % 2) * 256
            q_ps = psP.tile([64, width], fp32, tag="proj", name=f"qx{q}")
            for c in range(CCH):
                mi = nc.tensor.matmul(
                    q_ps, lhsT=w_sb[:, c, D : 2 * D],
                    rhs=slab[:, c, coff : coff + width],
                    start=(c == 0), stop=(c == CCH - 1),
                )
                if c == 0:
                    pe_after_sexp(mi, only=False)
            if q == 1:
                # chain-head quarter: the ACT engine is idle before the
                # first exp, so this copy runs parallel to the DVE copies
                nc.scalar.copy(qT[0:64, q * 256 : q * 256 + width], q_ps)
            else:
                nc.vector.tensor_copy(qT[0:64, q * 256 : q * 256 + width], q_ps)

        def proj_bot(si):
            """K-only projection for bottom slab si (si=0 -> slab2 ->
            A-bottom, si=1 -> slab3 -> B-bottom) into psum rows 64-127."""
            slab = xs[2 + si]
            k_ps = psP.tile([128, 512], fp32, tag="proj", name=f"kbps{si}")
            for c in range(CCH):
                mi = nc.tensor.matmul(
                    k_ps[64:128, :], lhsT=w_sb[:, c, 0:D], rhs=slab[:, c, :],
                    start=(c == 0), stop=(c == CCH - 1),
                )
                if c == 0:
                    pe_after_sexp(mi)
            proj_last[si] = mi
            nc.vector.tensor_copy(kstk[si][64:128, :], k_ps[64:128, :])

        vcopy_inst = {}

        def transp_half(si, half, only_p0=None):
            """V natural (+ones) for the 4 chunks of one half of stack si."""
            vo = vones[si]
            for p0 in ((0, 1) if only_p0 is None else (only_p0,)):
                pt2 = psP.tile(
                    [128, 128], bf16, tag="proj", name=f"tp{si}_{half}_{p0}"
                )
                for dk in range(2):
                    within = p0 * 2 + dk
                    nc.tensor.transpose(
                        pt2[:, dk * 64 : (dk + 1) * 64],
                        in_=kstk[si][64 * half : 64 * half + 64,
                                     within * CHUNK : (within + 1) * CHUNK],
                        identity=ident[64 * half : 64 * half + 64,
                                       64 * half : 64 * half + 64],
                    )
                w0 = half * 4 + p0 * 2
                ci = nc.vector.tensor_copy(vo[:, w0 : w0 + 2, 0:D], pt2)
                vcopy_inst[(si, half, p0 * 2)] = ci
                vcopy_inst[(si, half, p0 * 2 + 1)] = ci

        def lhsT_of(c):
            si, half, within = _chunk_stack(c)
            return kstk[si][64 * half : 64 * half + 64,
                            within * CHUNK : (within + 1) * CHUNK]

        def vones_of(c):
            si, half, within = _chunk_stack(c)
            return vones[si][:, half * 4 + within, :]

        pt_tiles = {}
        last_st = [None]
        proj_last = {}

        def pe_after_sexp(mi, only=False):
            # force this (first) proj matmul behind the latest S^T group in
            # the PE queue so the scheduler cannot starve the exp chain
            if only and last_st[0] is not None:
                add_dep_helper(mi.ins, last_st[0].ins, reason="chain first")

        def sexp_group(j, g):
            """S^T matmuls + exp (+ masks) for group g of slot j. For pair
            groups the issue order (own, oth, own, oth) row-packs each pair
            onto the two PE array halves concurrently; for 'mixed' the
            (0,2,1,3) order does the same."""
            gch, kind = _slot_groups_def(j)[g]
            s_ps = psS.tile([128, GROUP * QTILE], fp32, tag="s",
                            name=f"sps{j}_{g}")
            order = (0, 2, 1, 3) if kind == "mixed" else (0, 1, 2, 3)
            for sl in order:
                cc = gch[sl]
                half = _chunk_stack(cc)[1]
                last_st[0] = nc.tensor.matmul(
                    s_ps[:, sl * QTILE : (sl + 1) * QTILE],
                    lhsT=lhsT_of(cc),
                    rhs=qT[64 * half : 64 * half + 64,
                           j * QTILE : (j + 1) * QTILE],
                    start=True, stop=True,
                )
            pt = ptpool.tile([128, GROUP * QTILE], bf16, tag="pt", name=f"pt{j}_{g}")
            nc.scalar.activation(
                pt, s_ps, mybir.ActivationFunctionType.Exp, scale=SCALE
            )
            if kind == "pair_last":
                # slices 0,2 = own diagonal chunks 2j, 2j+1 -> causal masks;
                # slices 1,3 = oth tail chunks -> 0/1 role multiplier
                nc.vector.tensor_mul(
                    pt[:, 0:QTILE], pt[:, 0:QTILE], mask_sb[:, 0:QTILE]
                )
                nc.vector.tensor_mul(
                    pt[:, 2 * QTILE : 3 * QTILE], pt[:, 2 * QTILE : 3 * QTILE],
                    mask_sb[:, QTILE:],
                )
                nc.vector.tensor_scalar_mul(
                    pt[:, QTILE : 2 * QTILE], pt[:, QTILE : 2 * QTILE],
                    scal_sb[:, j : j + 1],
                )
                nc.vector.tensor_scalar_mul(
                    pt[:, 3 * QTILE :], pt[:, 3 * QTILE :],
                    scal_sb[:, j : j + 1],
                )
            elif kind == "mixed":
                nc.vector.tensor_mul(
                    pt[:, 0 : 2 * QTILE], pt[:, 0 : 2 * QTILE], mask_sb
                )
                nc.vector.tensor_scalar_mul(
                    pt[:, 2 * QTILE :], pt[:, 2 * QTILE :],
                    scal_sb[:, j : j + 1],
                )
            elif kind == "own_diag":
                nc.vector.tensor_mul(
                    pt[:, 2 * QTILE :], pt[:, 2 * QTILE :], mask_sb
                )
            elif kind == "oth_tail":
                nc.vector.tensor_scalar_mul(
                    pt[:, 2 * QTILE :], pt[:, 2 * QTILE :],
                    scal_sb[:, j : j + 1],
                )
            pt_tiles[(j, g)] = pt

        pv_done = {j: [] for j in range(N_SLOTS)}

        def pv_group(j, g, after=None):
            """PV for group g of slot j: a CLOSED 4-matmul psum chain (one
            full 2KB bank) + SBUF accumulate on vector. Ships the slot-pair
            DMA when the slot's last group lands."""
            gdefs = _slot_groups_def(j)
            ngroups = len(gdefs)
            gch, _ = gdefs[g]
            pt = pt_tiles.pop((j, g))
            g_ps = psO.tile([D + 1, 2 * QTILE], fp32, tag="o",
                            name=f"ops{j}_{g}")[:, 0:QTILE]
            for sl, cc in enumerate(gch):
                mi = nc.tensor.matmul(
                    g_ps, lhsT=vones_of(cc),
                    rhs=pt[:, sl * QTILE : (sl + 1) * QTILE],
                    start=(sl == 0), stop=(sl == len(gch) - 1),
                )
                if sl == 0 and after is not None:
                    # keep the PE scheduler from running this PV chain
                    # before the next projection (the chain feed) is done
                    add_dep_helper(mi.ins, after.ins, reason="PV after proj")
                # the strided partial-inner V-copy region is mistracked
                # by Tile's dep layer -- enforce copy -> PV-read
                # explicitly (arg order: first WAITS ON second)
                add_dep_helper(mi.ins, vcopy_inst[_chunk_stack(cc)].ins,
                               sync=True, reason="PV waits on V cols")
            if not pv_done[j]:
                nc.vector.tensor_copy(o_all[:, j, :], g_ps)
            else:
                nc.vector.tensor_add(o_all[:, j, :], o_all[:, j, :], g_ps)
            pv_done[j].append(g)
            if len(pv_done[j]) == ngroups:
                o_done[j] = True
                # ship each slot as soon as it completes; the final DMA
                # then carries only one slot's 66KB
                nc.sync.dma_start(
                    out=out[:, j * QTILE:(j + 1) * QTILE],
                    in_=o_all[:, j:j + 1, :],
                )

        # ---- emission order strictly follows slab arrival ----
        # phase 0: slab 0 (A-top + Q quarters 0-1)
        proj_top(0)
        proj_q(1)
        nc.sync.dma_start(out=qT[0:64, 0:QTILE], in_=qT[64:128, 0:QTILE])
        sexp_group(1, 0)   # {0,1,2,3} own_diag
        transp_half(0, 0)
        # phase 1: slab 1 (B-top + Q quarters 2-3, one 512-wide chain)
        proj_q(2, width=512)
        sexp_group(2, 0)   # {0..3}
        sexp_group(3, 0)   # {0..3}
        proj_top(1)
        sexp_group(3, 2)   # {4..7} own_diag
        transp_half(1, 0)
        pv_group(1, 0)
        # phase 2: slab 2 (A-bottom)
        proj_bot(0)
        sexp_group(0, 0)   # {0,1,8,9} mixed
        sexp_group(1, 1)   # {8..11} oth_tail
        sexp_group(2, 1)   # {8..11}
        sexp_group(3, 1)   # {8..11}
        transp_half(0, 1)
        pv_group(2, 0, after=proj_last[0])
        pv_group(3, 0, after=proj_last[0])
        pv_group(3, 2, after=proj_last[0])
        # phase 3: slab 3 (B-bottom)
        proj_bot(1)
        sexp_group(2, 2)   # {4,5,12,13} mixed
        sexp_group(3, 3)   # {12..15} oth_tail
        transp_half(1, 1)
        pv_group(0, 0, after=proj_last[1])
        pv_group(1, 1, after=proj_last[1])
        pv_group(2, 1, after=proj_last[1])
        pv_group(3, 1, after=proj_last[1])
        pv_group(2, 2, after=proj_last[1])
        pv_group(3, 3, after=proj_last[1])

    nc.compile()
    return nc


_NC_CACHE = None


def _get_nc():
    global _NC_CACHE
    if _NC_CACHE is None:
        _NC_CACHE = _build_graph()
    return _NC_CACHE


def _perm_tiles(r):
    """permuted 256-col tile order: own tiles (2j+r) first, then others."""
    own = [2 * j + r for j in range(N_SLOTS)]
    oth = [2 * j + (1 - r) for j in range(N_SLOTS)]
    return own + oth


def _host_prep(x, W_Q, W_K):
    in_maps = []
    w = np.concatenate([W_K.T, W_Q.T], axis=1).astype(BF16)  # [1024, 128]
    wkq = np.ascontiguousarray(w.reshape(8, 128, 2 * D).transpose(1, 0, 2))
    for i in range(N_CORES):
        b, r = i % B, i // B
        perm = _perm_tiles(r)
        xt = x[b].T.astype(BF16)  # [1024, 2048]
        cols = np.concatenate(
            [np.arange(QTILE * p, QTILE * p + QTILE) for p in perm]
        )
        xkt = xt[:, cols]
        # slab-major: [slab, part, cchunk, 512]
        x4 = np.ascontiguousarray(
            xkt.reshape(8, 128, 4, 512).transpose(2, 1, 0, 3)
        )
        # scal[p, j]: 0/1 multiplier for the other-side tail chunks
        # (r=1 -> fully valid, r=0 -> fully masked)
        sc = np.full((CHUNK, N_SLOTS), float(r), dtype=np.float32)
        in_maps.append(
            {
                "x4": x4,
                "wkq": wkq,
                "scal": np.ascontiguousarray(sc.astype(np.float32)),
            }
        )
    return in_maps


def _ensure_ntff_hook():
    """Install the antenv.axon_hooks shim so trace=True works under axon."""
    import types

    try:
        from antenv.axon_hooks import get_axon_ntff_profile_hook  # noqa: F401

        return
    except ImportError:
        pass
    import antenv

    mod = types.ModuleType("antenv.axon_hooks")
    mod._hook = None

    def set_axon_ntff_profile_hook(h):
        mod._hook = h

    def get_axon_ntff_profile_hook():
        return mod._hook

    mod.set_axon_ntff_profile_hook = set_axon_ntff_profile_hook
    mod.get_axon_ntff_profile_hook = get_axon_ntff_profile_hook
    sys.modules["antenv.axon_hooks"] = mod
    antenv.axon_hooks = mod
    try:
        from trn_agent_boot.trn_boot import _ntff_profile_via_ctypes

        hook = _ntff_profile_via_ctypes("/opt/axon/libaxon_pjrt.so")
        if hook is not None:
            set_axon_ntff_profile_hook(hook)
    except Exception as e:  # degrade to no tracing
        print(f"ntff hook install failed: {e}")


def kernel(x, W_Q, W_K, W_V=None, **_unused):
    global LAST_RESULTS
    if TRACE:
        _ensure_ntff_hook()
    x = np.asarray(x, dtype=np.float32)
    W_Q = np.asarray(W_Q, dtype=np.float32)
    W_K = np.asarray(W_K, dtype=np.float32)

    from concourse.bass_utils import run_bass_kernel_spmd

    nc = _get_nc()
    in_maps = _host_prep(x, W_Q, W_K)
    res = run_bass_kernel_spmd(
        nc,
        in_maps,
        core_ids=list(range(N_CORES)),
        trace=TRACE,
        trace_cores=TRACE_CORES,
    )
    LAST_RESULTS = res

    y = np.empty((B, T, D), dtype=np.float32)
    for i in range(N_CORES):
        b, r = i % B, i // B
        ot = res.results[i]["out"]  # [65, 1024]
        o = ot[0:D, :] / ot[D : D + 1, :]
        for j in range(N_SLOTS):
            t0 = QTILE * (2 * j + r)
            y[b, t0 : t0 + QTILE, :] = o[:, j * QTILE : (j + 1) * QTILE].T
    return y
